# revision 2
# baseline (speedup 1.0000x reference)
"""DotProductGraphAttention Trainium2 kernel.

Reference computation (per batch b, head h):
    S = Q @ K^T / 8                      [N, N]
    P = softmax(where(adj > 0, S, -inf), axis=-1)
    O = P @ V                            [N, D]
Output: h_prime[B,H,N,D].reshape(N, B, H, D)  (flat reshape)

Softmax is computed max-free (scores are O(1): S ~ N(0,1), exp never
overflows fp32):  P = exp(S/8) * A;  O = (P @ V) / rowsum(P), with rowsum
obtained by augmenting V with a leading ones column.

Sharding: 8 cores = (batch b in 0..3) x (query half in 0..1). Each core owns
all 8 heads for its (b, 1024-query slice): K/V per head are full [2048, 64],
the adj row-slice [1024, 2048] is shared by all heads on the core.

Per-core pipeline (matmul operands bf16, accumulation fp32):
  - adj:  SWDGE cast-DMA i32->bf16 to HBM scratch, then HWDGE transpose-DMA
          into a resident A^T sbuf tile [128, 16 jt, 1024 i].
  - Q,K:  SWDGE cast-DMA f32->bf16 to sbuf, PE-transposed into K^T/Q^T with
          even j-tiles on partitions 0-63 and odd on 64-127 (row-tiled QK).
  - S^T:  row-tiled matmul pairs (contraction d=64 on each partition half)
          into a 6-bank PSUM ring of [128, 512] blocks.
  - P^T:  ScalarE exp(0.125 * S) reading up to 3 ring banks per op -> bf16;
          VectorE tensor_tensor mult with A^T (bf16 2x mode).
  - O:    PV matmuls, lhsT = P^T block [128,128], rhs = V' = [1|V] [128,65],
          16-j accumulation groups per i-tile; psum col 0 is the rowsum.
          PV of head h-1 is interleaved between head h's QK/exp windows so
          TensorE never blocks ScalarE (engines execute their FIFO in
          program order).
  - out:  reciprocal + broadcast multiply, DMA to HBM.
"""

import sys

if "/opt/trn_rl_repo" not in sys.path:
    sys.path.insert(0, "/opt/trn_rl_repo")

from contextlib import ExitStack

import numpy as np

import concourse.bacc as bacc
import concourse.mybir as mybir
import concourse.tile as tile
from concourse.masks import make_identity

B, H, N, D = 4, 8, 2048, 64
NCORES = 8
QH = N // 2  # queries per core (1024)
NJT = N // 128  # 16 key tiles
NIT = QH // 128  # 8 query tiles per core
RING = 6  # S^T psum ring banks
WIN = 3  # ring banks per exp op
BF16 = mybir.dt.bfloat16
F32 = mybir.dt.float32

_CACHED_NC = {}


def build_nc(replay: int = 1):
    """Build + compile the per-core Bass program (same NEFF on all 8 cores)."""
    if replay in _CACHED_NC:
        return _CACHED_NC[replay]

    nc = bacc.Bacc("TRN2", target_bir_lowering=False, debug=False)
    q_h = nc.dram_tensor("q_bh", [H, QH, D], F32, kind="ExternalInput")
    k_h = nc.dram_tensor("k_bh", [H, N, D], F32, kind="ExternalInput")
    v_h = nc.dram_tensor("v_bh", [H, N, D], F32, kind="ExternalInput")
    adj_h = nc.dram_tensor("adj_s", [QH, N], mybir.dt.int32, kind="ExternalInput")
    out_h = nc.dram_tensor("out", [H, QH, D], F32, kind="ExternalOutput")
    adj_scr = nc.dram_tensor("adj_scr", [QH, N], BF16, kind="Internal")

    with tile.TileContext(nc) as tc, ExitStack() as ctx:
        singles = ctx.enter_context(tc.tile_pool(name="singles", bufs=1))
        io = ctx.enter_context(tc.tile_pool(name="io", bufs=2))
        ptp = ctx.enter_context(tc.tile_pool(name="ptp", bufs=2))
        kqp = ctx.enter_context(tc.tile_pool(name="kqp", bufs=2))
        outp = ctx.enter_context(tc.tile_pool(name="outp", bufs=3))
        ps_ring = ctx.enter_context(tc.tile_pool(name="psring", bufs=1, space="PSUM"))
        ps_o = ctx.enter_context(tc.tile_pool(name="pso", bufs=1, space="PSUM"))
        ps_tr = ctx.enter_context(tc.tile_pool(name="pstr", bufs=1, space="PSUM"))

        ident = singles.tile([128, 128], BF16)
        make_identity(nc, ident[:])
        at = singles.tile([128, NJT, QH], BF16, tag="at")
        s_ring = ps_ring.tile([128, RING, 512], F32, tag="sring")

        def emit_adj_prep():
            # cast adj rows to bf16 scratch, transpose into resident A^T:
            # at[j_in_tile(p), jt, i] = adj[i, jt*128 + j_in_tile]
            NRB = 4  # row blocks, pipelined with the per-block transposes
            rb = QH // NRB
            for r in range(NRB):
                nc.gpsimd.dma_start(
                    out=adj_scr[r * rb : (r + 1) * rb, :],
                    in_=adj_h[r * rb : (r + 1) * rb, :],
                )
                for j in range(NJT):
                    nc.sync.dma_start(
                        out=at[:, j, r * rb : (r + 1) * rb],
                        in_=adj_scr[r * rb : (r + 1) * rb, j * 128 : (j + 1) * 128],
                        transpose=True,
                    )

        def emit_head_front(h):
            """Loads + PE transposes + QK/exp/mask windows for head h.

            Yields between windows so the caller can interleave the previous
            head's PV work into the TensorE stream. Returns (pt, vp) tiles.
            """
            kn = io.tile([128, NJT, D], BF16, tag="kn")
            nc.gpsimd.dma_start(
                out=kn[:], in_=k_h[h].rearrange("(j p) d -> p j d", p=128)
            )
            qn = io.tile([128, NIT, D], BF16, tag="qn")
            nc.gpsimd.dma_start(
                out=qn[:], in_=q_h[h].rearrange("(i p) d -> p i d", p=128)
            )
            vp = io.tile([128, NJT, D + 1], BF16, tag="vp")
            nc.vector.memset(vp[:, :, 0:1], 1.0)
            nc.gpsimd.dma_start(
                out=vp[:, :, 1:], in_=v_h[h].rearrange("(j p) d -> p j d", p=128)
            )

            # K^T: even jt -> partitions 0:64, odd jt -> 64:128 (row tiling)
            kt = kqp.tile([128, NJT // 2, 128], BF16, tag="kt")
            tp = ps_tr.tile([128, 8, 128], BF16, tag="tp")
            for s in range(NJT // 2):
                nc.tensor.transpose(tp[0:64, s, :], kn[:, 2 * s, :], ident[:])
                nc.tensor.transpose(tp[64:128, s, :], kn[:, 2 * s + 1, :], ident[:])
            nc.vector.tensor_copy(kt[:], tp[:])
            yield

            # Q^T replicated on both partition halves
            qt = kqp.tile([128, NIT, 128], BF16, tag="qt")
            tq = ps_tr.tile([128, 8, 128], BF16, tag="tp")
            for i in range(NIT):
                nc.tensor.transpose(tq[0:64, i, :], qn[:, i, :], ident[:])
                nc.tensor.transpose(tq[64:128, i, :], qn[:, i, :], ident[:])
            nc.vector.tensor_copy(qt[:], tq[:])
            yield

            pt = ptp.tile([128, NJT, QH], BF16, tag="pt")
            self_state = {"pt": pt, "vp": vp}
            yield self_state

            slot = 0  # global ring position (reset per head keeps windows aligned)
            for isup in range(2):
                isl = slice(512 * isup, 512 * (isup + 1))
                j = 0
                while j < NJT:
                    base = slot % RING
                    width = min(WIN, NJT - j, RING - base)
                    for g in range(width):
                        jj = j + g
                        half = jj % 2
                        nc.tensor.matmul(
                            s_ring[:, base + g, :],
                            lhsT=kt[64 * half : 64 * half + 64, jj // 2, :],
                            rhs=qt[
                                64 * half : 64 * half + 64,
                                4 * isup : 4 * isup + 4,
                                :,
                            ],
                            start=True,
                            stop=True,
                        )
                    nc.scalar.activation(
                        out=pt[:, j : j + width, isl],
                        in_=s_ring[:, base : base + width, :],
                        func=mybir.ActivationFunctionType.Exp,
                        scale=0.125,
                    )
                    nc.vector.tensor_tensor(
                        out=pt[:, j : j + width, isl],
                        in0=pt[:, j : j + width, isl],
                        in1=at[:, j : j + width, isl],
                        op=mybir.AluOpType.mult,
                    )
                    slot += width
                    j += width
                    yield

        def emit_pv(h, pt, vp):
            """PV + normalize + store for head h, as an interleavable generator."""
            for grp in range(NIT // 4):
                opsum = ps_o.tile([128, 4, D + 1], F32, tag="opsum")
                for it in range(4 * grp, 4 * grp + 4):
                    for j in range(NJT):
                        nc.tensor.matmul(
                            opsum[:, it % 4, :],
                            lhsT=pt[:, j, it * 128 : (it + 1) * 128],
                            rhs=vp[:, j, :],
                            start=(j == 0),
                            stop=(j == NJT - 1),
                        )
                    yield
                rr = outp.tile([128, 4, 1], F32, tag="rr")
                nc.vector.reciprocal(out=rr[:], in_=opsum[:, :, 0:1])
                o_sb = outp.tile([128, 4, D], F32, tag="osb")
                nc.vector.tensor_tensor(
                    out=o_sb[:],
                    in0=opsum[:, :, 1:],
                    in1=rr[:, :, 0:1].to_broadcast([128, 4, D]),
                    op=mybir.AluOpType.mult,
                )
                nc.sync.dma_start(
                    out=out_h[h, 512 * grp : 512 * (grp + 1), :].rearrange(
                        "(i p) d -> p i d", p=128
                    ),
                    in_=o_sb[:],
                )
                yield

        for rep in range(replay):
            emit_adj_prep()
            prev_pv = iter(())
            for h in range(H):
                front = emit_head_front(h)
                state = None
                for step in front:
                    if isinstance(step, dict):
                        state = step
                        continue
                    next(prev_pv, None)  # interleave previous head's PV work
                for _ in prev_pv:  # drain any leftovers
                    pass
                prev_pv = emit_pv(h, state["pt"], state["vp"])
            for _ in prev_pv:  # last head's PV
                pass

    nc.compile()
    _CACHED_NC[replay] = nc
    return nc


def shard_inputs(queries, keys, values, adj):
    """Per-core input dicts: core c -> (batch c%4, query half c//4)."""
    in_maps = []
    for c in range(NCORES):
        b, qh = c % B, c // B
        in_maps.append(
            {
                "q_bh": np.ascontiguousarray(queries[b, :, qh * QH : (qh + 1) * QH, :]),
                "k_bh": np.ascontiguousarray(keys[b]),
                "v_bh": np.ascontiguousarray(values[b]),
                "adj_s": np.ascontiguousarray(adj[qh * QH : (qh + 1) * QH, :]),
            }
        )
    return in_maps


def assemble_output(results):
    h_prime = np.empty((B, H, N, D), dtype=np.float32)
    for c in range(NCORES):
        b, qh = c % B, c // B
        h_prime[b, :, qh * QH : (qh + 1) * QH, :] = results[c]["out"]
    return h_prime.reshape(N, B, H, D)


def kernel(queries, keys, values, adj):
    queries = np.asarray(queries, dtype=np.float32)
    keys = np.asarray(keys, dtype=np.float32)
    values = np.asarray(values, dtype=np.float32)
    adj = np.asarray(adj, dtype=np.int32)

    from concourse.bass_utils import run_bass_kernel_spmd

    nc = build_nc()
    res = run_bass_kernel_spmd(
        nc, shard_inputs(queries, keys, values, adj), core_ids=list(range(NCORES))
    )
    return assemble_output(res.results)


# revision 15
# speedup vs baseline: 1.1788x; 1.1788x over previous
"""DotProductGraphAttention Trainium2 kernel.

Reference computation (per batch b, head h):
    S = Q @ K^T / 8                      [N, N]
    P = softmax(where(adj > 0, S, -inf), axis=-1)
    O = P @ V                            [N, D]
Output: h_prime[B,H,N,D].reshape(N, B, H, D)  (flat reshape)

Softmax is computed max-free (S ~ N(0,1); exp never overflows fp32):
    P = exp(S/8) * A;  O = (P @ V) / rowsum(P)
with the rowsum obtained by augmenting V with a leading ones column.

Sharding: 8 cores = (batch b in 0..3) x (query half in 0..1). Each core owns
all 8 heads for its (b, 1024-query slice): K/V per head are full [2048, 64],
the adj row-slice [1024, 2048] is shared by all heads on the core.

Per-core pipeline (matmul operands bf16, accumulation fp32):
  - adj:  per key-tile j: SWDGE cast-DMA i32->bf16 [1024,128] to HBM scratch,
          HWDGE transpose-DMA into a resident A^T sbuf tile [128, jt, 1024].
  - Q,K:  SWDGE cast-DMA f32->bf16 to sbuf; PE transposes two 64-wide tiles
          at a time ([128,128] one-shot): K^T with even j-tiles on partitions
          0-63 / odd on 64-127 (row-tiled QK), Q^T replicated on both halves
          via a stride-0 doubled access pattern.
  - S^T:  per slot (j, isup): matmul (d=64 contraction on alternating
          partition halves -> concurrent row groups) into a 5-bank PSUM ring.
  - P^T:  ScalarE exp(0.125*S) over up to 3 contiguous ring banks -> bf16 at
          pt flat offsets (slot order makes windows contiguous);
          VectorE tensor_tensor mult with A^T (bf16 2x mode).
  - O^T:  PV matmuls with stationary V' = [1|V] (16 LDWEIGHTS per head
          instead of 128): out [65, 512] psum accumulated over j per query
          half; col... row 0 is the rowsum. Copied to sbuf (fp32), PE
          back-transposed per 128-query tile into [128, 4, 65] psum,
          then reciprocal + broadcast-multiply normalize, DMA to HBM.
  - PV/normalize of head h-1 is interleaved between head h's QK/exp windows
    so no engine FIFO blocks another engine's producer.
"""

import sys

if "/opt/trn_rl_repo" not in sys.path:
    sys.path.insert(0, "/opt/trn_rl_repo")

from contextlib import ExitStack

import numpy as np

import concourse.bacc as bacc
import concourse.mybir as mybir
import concourse.tile as tile
from concourse.masks import make_identity

B, H, N, D = 4, 8, 2048, 64
NCORES = 8
QH = N // 2  # queries per core (1024)
NJT = N // 128  # 16 key tiles
NIT = QH // 128  # 8 query tiles per core
RING = 5  # S^T psum ring banks
WIN = 3  # max ring banks per exp op
BF16 = mybir.dt.bfloat16
F32 = mybir.dt.float32

_CACHED_NC = {}


def build_nc(replay: int = 1):
    """Build + compile the per-core Bass program (same NEFF on all 8 cores)."""
    if replay in _CACHED_NC:
        return _CACHED_NC[replay]

    nc = bacc.Bacc("TRN2", target_bir_lowering=False, debug=False)
    q_h = nc.dram_tensor("q_bh", [H, QH, D], F32, kind="ExternalInput")
    k_h = nc.dram_tensor("k_bh", [H, N, D], F32, kind="ExternalInput")
    v_h = nc.dram_tensor("v_bh", [H, N, D], F32, kind="ExternalInput")
    adj_h = nc.dram_tensor("adj_s", [QH, N], mybir.dt.int32, kind="ExternalInput")
    out_h = nc.dram_tensor("out", [H, QH, D], F32, kind="ExternalOutput")
    adj_scr = nc.dram_tensor("adj_scr", [QH, N], BF16, kind="Internal")

    with tile.TileContext(nc) as tc, ExitStack() as ctx:
        singles = ctx.enter_context(tc.tile_pool(name="singles", bufs=1))
        io = ctx.enter_context(tc.tile_pool(name="io", bufs=2))
        ptp = ctx.enter_context(tc.tile_pool(name="ptp", bufs=2))
        kqp = ctx.enter_context(tc.tile_pool(name="kqp", bufs=2))
        otp = ctx.enter_context(tc.tile_pool(name="otp", bufs=2))
        outp = ctx.enter_context(tc.tile_pool(name="outp", bufs=3))
        ps_ring = ctx.enter_context(tc.tile_pool(name="psring", bufs=1, space="PSUM"))
        ps_ot = ctx.enter_context(tc.tile_pool(name="psot", bufs=1, space="PSUM"))
        ps_tr = ctx.enter_context(tc.tile_pool(name="pstr", bufs=1, space="PSUM"))

        ident = singles.tile([128, 128], BF16)
        make_identity(nc, ident[:])
        at = singles.tile([128, NJT, QH], BF16, tag="at")
        at_flat = at[:].rearrange("p a b -> p (a b)")
        s_ring = ps_ring.tile([128, RING, 512], F32, tag="sring")
        ring_pos = {"slot": 0}

        def emit_loads(h):
            kn = io.tile([128, NJT, D], BF16, tag="kn")
            nc.gpsimd.dma_start(
                out=kn[:], in_=k_h[h].rearrange("(j p) d -> p j d", p=128)
            )
            qn = io.tile([128, NIT, D], BF16, tag="qn")
            nc.gpsimd.dma_start(
                out=qn[:], in_=q_h[h].rearrange("(i p) d -> p i d", p=128)
            )
            vp = io.tile([128, NJT, D + 2], BF16, tag="vp")  # 66-wide: 4B-aligned j slices
            nc.vector.memset(vp[:, :, D : D + 1], 1.0)
            nc.gpsimd.dma_start(
                out=vp[:, :, 0:D], in_=v_h[h].rearrange("(j p) d -> p j d", p=128)
            )
            return kn, qn, vp

        def emit_adj_prep():
            """Cast + transpose adj in 2-key-tile chunks; interleavable."""
            for c in range(NJT // 2):
                cs = slice(c * 256, (c + 1) * 256)
                nc.gpsimd.dma_start(out=adj_scr[:, cs], in_=adj_h[:, cs])
                for j in (2 * c, 2 * c + 1):
                    js = slice(j * 128, (j + 1) * 128)
                    nc.sync.dma_start(
                        out=at[:, j, :], in_=adj_scr[:, js], transpose=True
                    )
                yield

        def emit_transposes(kn, qn):
            # K^T: one [128,128] transpose per pair of 64-wide K tiles lands
            # even tiles on partitions 0-63 and odd on 64-127.
            kt = kqp.tile([128, NJT // 2, 128], BF16, tag="kt")
            tp = ps_tr.tile([128, 8, 128], BF16, tag="tp")
            for s in range(NJT // 2):
                nc.tensor.transpose(tp[:, s, :], kn[:, 2 * s : 2 * s + 2, :], ident[:])
            nc.vector.tensor_copy(kt[:], tp[:])
            yield
            # Q^T: stride-0 doubling replicates each tile on both halves.
            qt = kqp.tile([128, NIT, 128], BF16, tag="qt")
            tq = ps_tr.tile([128, 8, 128], BF16, tag="tp")
            for i in range(NIT):
                nc.tensor.transpose(tq[0:D, i, :], qn[:, i, :], ident[:])
                nc.tensor.transpose(tq[D : 2 * D, i, :], qn[:, i, :], ident[:])
            nc.vector.tensor_copy(qt[:], tq[:])
            yield (kt, qt)

        def emit_windows(h, kt, qt):
            """QK -> exp -> mask in ring windows; yields after each window."""
            pt = ptp.tile([128, NJT * QH], BF16, tag="pt")  # flat [j, isup, 512]
            yield pt
            # slots in (j outer, isup inner) order -> pt offsets are contiguous
            slots = [(j, isup) for j in range(NJT) for isup in range(2)]
            w = 0
            while w < len(slots):
                base = ring_pos["slot"] % RING
                width = min(WIN, len(slots) - w, RING - base)
                for g, (j, isup) in enumerate(slots[w : w + width]):
                    half = j % 2
                    nc.tensor.matmul(
                        s_ring[:, base + g, :],
                        lhsT=kt[64 * half : 64 * half + 64, j // 2, :],
                        rhs=qt[64 * half : 64 * half + 64, 4 * isup : 4 * isup + 4, :],
                        start=True,
                        stop=True,
                    )
                j0, isup0 = slots[w]
                off = (2 * j0 + isup0) * 512
                nc.scalar.activation(
                    out=pt[:, off : off + width * 512],
                    in_=s_ring[:, base : base + width, :].rearrange(
                        "p a b -> p (a b)"
                    ),
                    func=mybir.ActivationFunctionType.Exp,
                    scale=0.125,
                )
                nc.vector.tensor_tensor(
                    out=pt[:, off : off + width * 512],
                    in0=pt[:, off : off + width * 512],
                    in1=at_flat[:, off : off + width * 512],
                    op=mybir.AluOpType.mult,
                )
                ring_pos["slot"] += width
                w += width
                yield

        def emit_pv(h, pt, vp):
            """O^T = V'^T P^T per query half; back-transpose; normalize; store.

            Both halves' matmuls run first (two psum banks), evacuation
            follows, so no PE instruction ever waits on a just-emitted DVE op.
            """
            ptv = pt.rearrange("p (j i) -> p j i", j=NJT)
            ot_sbs = []
            for ihalf in range(2):
                ot_ps = ps_ot.tile([65, 512], F32, tag=f"ot{ihalf}")
                for j in range(NJT):
                    nc.tensor.matmul(
                        ot_ps[:, :],
                        lhsT=vp[:, j, 0 : D + 1],
                        rhs=ptv[:, j, 512 * ihalf : 512 * (ihalf + 1)],
                        start=(j == 0),
                        stop=(j == NJT - 1),
                    )
                    if j % 4 == 3:
                        yield
                ot_sb = otp.tile([65, 512], BF16, tag=f"otsb{ihalf}")
                nc.vector.tensor_copy(ot_sb[:], ot_ps[:])
                ot_sbs.append(ot_sb)
                yield
            for ihalf in range(2):
                ob = ps_tr.tile([128, 4, D + 2], BF16, tag="tp")  # 66-wide: aligned slices
                for itl in range(4):
                    nc.tensor.transpose(
                        ob[:, itl, 0 : D + 1],
                        ot_sbs[ihalf][:, itl * 128 : (itl + 1) * 128],
                        ident[0:65, 0:65],
                    )
                yield
                rr = outp.tile([128, 4, 1], F32, tag="rr")
                nc.vector.reciprocal(out=rr[:], in_=ob[:, :, D : D + 1])
                o_sb = outp.tile([128, 4, D], F32, tag="osb")
                nc.vector.tensor_tensor(
                    out=o_sb[:],
                    in0=ob[:, :, 0:D],
                    in1=rr[:, :, 0:1].to_broadcast([128, 4, D]),
                    op=mybir.AluOpType.mult,
                )
                nc.sync.dma_start(
                    out=out_h[h, 512 * ihalf : 512 * (ihalf + 1), :].rearrange(
                        "(i p) d -> p i d", p=128
                    ),
                    in_=o_sb[:],
                )
                yield

        def drive(front, prev_pv, per_step=1):
            out = None
            for step in front:
                if step is not None:
                    out = step
                    continue
                for _ in range(per_step):
                    next(prev_pv, None)
            return out

        for rep in range(replay):
            prev_pv = None
            for h in range(H):
                kn, qn, vp = emit_loads(h)
                if prev_pv is None:
                    prev_pv = emit_adj_prep()  # head 0 interleaves adj prep
                kt, qt = drive(emit_transposes(kn, qn), prev_pv)
                front = emit_windows(h, kt, qt)
                pt = next(front)
                for _ in front:
                    next(prev_pv, None)
                    next(prev_pv, None)
                for _ in prev_pv:
                    pass
                prev_pv = emit_pv(h, pt, vp)
            for _ in prev_pv:
                pass

    nc.compile()
    _CACHED_NC[replay] = nc
    return nc


def shard_inputs(queries, keys, values, adj):
    """Per-core input dicts: core c -> (batch c%4, query half c//4)."""
    in_maps = []
    for c in range(NCORES):
        b, qh = c % B, c // B
        in_maps.append(
            {
                "q_bh": np.ascontiguousarray(queries[b, :, qh * QH : (qh + 1) * QH, :]),
                "k_bh": np.ascontiguousarray(keys[b]),
                "v_bh": np.ascontiguousarray(values[b]),
                "adj_s": np.ascontiguousarray(adj[qh * QH : (qh + 1) * QH, :]),
            }
        )
    return in_maps


def assemble_output(results):
    h_prime = np.empty((B, H, N, D), dtype=np.float32)
    for c in range(NCORES):
        b, qh = c % B, c // B
        h_prime[b, :, qh * QH : (qh + 1) * QH, :] = results[c]["out"]
    return h_prime.reshape(N, B, H, D)


def kernel(queries, keys, values, adj):
    queries = np.asarray(queries, dtype=np.float32)
    keys = np.asarray(keys, dtype=np.float32)
    values = np.asarray(values, dtype=np.float32)
    adj = np.asarray(adj, dtype=np.int32)

    from concourse.bass_utils import run_bass_kernel_spmd

    nc = build_nc()
    res = run_bass_kernel_spmd(
        nc, shard_inputs(queries, keys, values, adj), core_ids=list(range(NCORES))
    )
    return assemble_output(res.results)


# revision 24
# speedup vs baseline: 1.4847x; 1.2595x over previous
"""DotProductGraphAttention Trainium2 kernel.

Reference computation (per batch b, head h):
    S = Q @ K^T / 8                      [N, N]
    P = softmax(where(adj > 0, S, -inf), axis=-1)
    O = P @ V                            [N, D]
Output: h_prime[B,H,N,D].reshape(N, B, H, D)  (flat reshape)

Softmax is computed max-free (S ~ N(0,1); exp never overflows fp32):
    P = exp(S/8) * A;  O = (P @ V) / rowsum(P)
with the rowsum obtained by augmenting V with a leading ones column.

Sharding: 8 cores = (batch b in 0..3) x (query half in 0..1). Each core owns
all 8 heads for its (b, 1024-query slice): K/V per head are full [2048, 64],
the adj row-slice [1024, 2048] is shared by all heads on the core.

Per-core pipeline (matmul operands bf16, accumulation fp32):
  - adj:  per key-tile j: SWDGE cast-DMA i32->bf16 [1024,128] to HBM scratch,
          HWDGE transpose-DMA into a resident A^T sbuf tile [128, jt, 1024].
  - Q,K:  SWDGE cast-DMA f32->bf16 to sbuf; PE transposes two 64-wide tiles
          at a time ([128,128] one-shot): K^T with even j-tiles on partitions
          0-63 / odd on 64-127 (row-tiled QK), Q^T replicated on both halves
          via a stride-0 doubled access pattern.
  - S^T:  per slot (j, isup): matmul (d=64 contraction on alternating
          partition halves -> concurrent row groups) into a 5-bank PSUM ring.
  - P^T:  ScalarE exp(0.125*S) over up to 3 contiguous ring banks -> bf16 at
          pt flat offsets (slot order makes windows contiguous);
          VectorE tensor_tensor mult with A^T (bf16 2x mode).
  - O^T:  PV matmuls with stationary V' = [1|V] (16 LDWEIGHTS per head
          instead of 128): out [65, 512] psum accumulated over j per query
          half; col... row 0 is the rowsum. Copied to sbuf (fp32), PE
          back-transposed per 128-query tile into [128, 4, 65] psum,
          then reciprocal + broadcast-multiply normalize, DMA to HBM.
  - PV/normalize of head h-1 is interleaved between head h's QK/exp windows
    so no engine FIFO blocks another engine's producer.
"""

import sys

if "/opt/trn_rl_repo" not in sys.path:
    sys.path.insert(0, "/opt/trn_rl_repo")

from contextlib import ExitStack

import numpy as np

import concourse.bacc as bacc
import concourse.mybir as mybir
import concourse.tile as tile
from concourse.masks import make_identity
from concourse.tile_rust import add_dep_helper

B, H, N, D = 4, 8, 2048, 64
NCORES = 8
QH = N // 2  # queries per core (1024)
NJT = N // 128  # 16 key tiles
NIT = QH // 128  # 8 query tiles per core
NWIN = 2  # rotating S^T window tiles (2 psum banks each)
WIN = 2  # banks (slots) per window
BF16 = mybir.dt.bfloat16
F32 = mybir.dt.float32

_CACHED_NC = {}


def build_nc(replay: int = 1):
    """Build + compile the per-core Bass program (same NEFF on all 8 cores)."""
    if replay in _CACHED_NC:
        return _CACHED_NC[replay]

    nc = bacc.Bacc("TRN2", target_bir_lowering=False, debug=False)
    q_h = nc.dram_tensor("q_bh", [H, QH, D], F32, kind="ExternalInput")
    k_h = nc.dram_tensor("k_bh", [H, N, D], F32, kind="ExternalInput")
    v_h = nc.dram_tensor("v_bh", [H, N, D], F32, kind="ExternalInput")
    adj_h = nc.dram_tensor("adj_s", [QH, N], mybir.dt.int32, kind="ExternalInput")
    out_h = nc.dram_tensor("out", [H, QH, D], F32, kind="ExternalOutput")
    adj_scr = nc.dram_tensor("adj_scr", [QH, N], BF16, kind="Internal")

    with tile.TileContext(nc) as tc, ExitStack() as ctx:
        singles = ctx.enter_context(tc.tile_pool(name="singles", bufs=1))
        io = ctx.enter_context(tc.tile_pool(name="io", bufs=2))
        ptp = ctx.enter_context(tc.tile_pool(name="ptp", bufs=2))
        kqp = ctx.enter_context(tc.tile_pool(name="kqp", bufs=2))
        otp = ctx.enter_context(tc.tile_pool(name="otp", bufs=2))
        outp = ctx.enter_context(tc.tile_pool(name="outp", bufs=3))
        ps_ring = ctx.enter_context(tc.tile_pool(name="psring", bufs=NWIN, space="PSUM"))
        ps_ot = ctx.enter_context(tc.tile_pool(name="psot", bufs=2, space="PSUM"))
        ps_tr = ctx.enter_context(tc.tile_pool(name="pstr", bufs=2, space="PSUM"))

        ident = singles.tile([128, 128], BF16)
        make_identity(nc, ident[:])
        at = singles.tile([128, NJT, QH], BF16, tag="at")
        at_flat = at[:].rearrange("p a b -> p (a b)")
        last_mask = {"ins": None}

        def emit_loads(h):
            kn = io.tile([128, NJT, D], BF16, tag="kn")
            nc.gpsimd.dma_start(
                out=kn[:], in_=k_h[h].rearrange("(j p) d -> p j d", p=128)
            )
            qn = io.tile([128, NIT, D], BF16, tag="qn")
            nc.gpsimd.dma_start(
                out=qn[:], in_=q_h[h].rearrange("(i p) d -> p i d", p=128)
            )
            vp = io.tile([128, NJT, D + 2], BF16, tag="vp")  # 66-wide: 4B-aligned j slices
            nc.vector.memset(vp[:, :, D : D + 1], 1.0)
            nc.gpsimd.dma_start(
                out=vp[:, :, 0:D], in_=v_h[h].rearrange("(j p) d -> p j d", p=128)
            )
            return kn, qn, vp

        def emit_adj_prep():
            """Cast + transpose adj. All DMAs issued up front: the SDMA
            engines pipeline transfers across queues, and masks consume the
            A^T tiles progressively."""
            for c in range(NJT // 4):
                cs = slice(c * 512, (c + 1) * 512)
                nc.gpsimd.dma_start(out=adj_scr[:, cs], in_=adj_h[:, cs])
                for j in range(4 * c, 4 * c + 4):
                    js = slice(j * 128, (j + 1) * 128)
                    nc.sync.dma_start(
                        out=at[:, j, :], in_=adj_scr[:, js], transpose=True
                    )

        def emit_transposes(kn, qn):
            # K^T: one [128,128] transpose per pair of 64-wide K tiles lands
            # even tiles on partitions 0-63 and odd on 64-127.
            kt = kqp.tile([128, NJT // 2, 128], BF16, tag="kt")
            tp = ps_tr.tile([128, 8, 128], BF16, tag="tp")
            for s in range(NJT // 2):
                nc.tensor.transpose(tp[:, s, :], kn[:, 2 * s : 2 * s + 2, :], ident[:])
            nc.vector.tensor_copy(kt[:], tp[:])
            yield
            # Q^T replicated on both partition halves.
            qt = kqp.tile([128, NIT, 128], BF16, tag="qt")
            tq = ps_tr.tile([128, 8, 128], BF16, tag="tp")
            for i in range(NIT):
                nc.tensor.transpose(tq[0:D, i, :], qn[:, i, :], ident[:])
                nc.tensor.transpose(tq[D : 2 * D, i, :], qn[:, i, :], ident[:])
            nc.vector.tensor_copy(qt[:], tq[:])
            yield (kt, qt)

        def emit_windows(h, kt, qt):
            """QK -> exp -> mask in ring windows; yields after each window."""
            pt = ptp.tile([128, NJT * QH], BF16, tag="pt")  # flat [j, isup, 512]
            yield pt
            # slots in (j outer, isup inner) order -> pt offsets are contiguous
            slots = [(j, isup) for j in range(NJT) for isup in range(2)]
            for w in range(0, len(slots), WIN):
                # each window gets its own psum tile so the WAR against the
                # window's exp is tracked per-tile (pool rotation = lookahead)
                sp = ps_ring.tile([128, WIN, 512], F32, tag="sring")
                for g, (j, isup) in enumerate(slots[w : w + WIN]):
                    half = j % 2
                    nc.tensor.matmul(
                        sp[:, g, :],
                        lhsT=kt[64 * half : 64 * half + 64, j // 2, :],
                        rhs=qt[64 * half : 64 * half + 64, 4 * isup : 4 * isup + 4, :],
                        start=True,
                        stop=True,
                    )
                j0, isup0 = slots[w]
                off = (2 * j0 + isup0) * 512
                nc.scalar.activation(
                    out=pt[:, off : off + WIN * 512],
                    in_=sp[:].rearrange("p a b -> p (a b)"),
                    func=mybir.ActivationFunctionType.Exp,
                    scale=0.125,
                )
                tt = nc.vector.tensor_tensor(
                    out=pt[:, off : off + WIN * 512],
                    in0=pt[:, off : off + WIN * 512],
                    in1=at_flat[:, off : off + WIN * 512],
                    op=mybir.AluOpType.mult,
                )
                last_mask["ins"] = tt.ins
                yield

        def emit_pv(h, pt, vp, after_ins):
            """O^T = V'^T P^T per query half; back-transpose; normalize; store.

            Both halves' matmuls run first (two psum banks), evacuation
            follows. The first matmul of each accumulation group carries an
            order-only dep on the head's last mask so the scheduler cannot
            hoist PV into the head's own window region (where it would stall
            on in-flight masks and head-of-line-block the QK stream).
            """
            ptv = pt.rearrange("p (j i) -> p j i", j=NJT)
            ot_sbs = []
            for ihalf in range(2):
                ot_ps = ps_ot.tile([65, 512], F32, tag="ot")
                for j in range(NJT):
                    mm = nc.tensor.matmul(
                        ot_ps[:, :],
                        lhsT=vp[:, j, 0 : D + 1],
                        rhs=ptv[:, j, 512 * ihalf : 512 * (ihalf + 1)],
                        start=(j == 0),
                        stop=(j == NJT - 1),
                    )
                    if j == 0 and after_ins is not None:
                        add_dep_helper(
                            mm.ins, after_ins, reason="pv after all masks"
                        )
                    if j % 4 == 3:
                        yield
                ot_sb = otp.tile([65, 512], BF16, tag=f"otsb{ihalf}")
                nc.vector.tensor_copy(ot_sb[:], ot_ps[:])
                ot_sbs.append(ot_sb)
                yield
                yield  # emission distance: next psum user waits on this copy
                yield
            for ihalf in range(2):
                ob = ps_ot.tile([128, 4, D + 2], BF16, tag="ot")  # aligned slices
                for itl in range(4):
                    nc.tensor.transpose(
                        ob[:, itl, 0 : D + 1],
                        ot_sbs[ihalf][:, itl * 128 : (itl + 1) * 128],
                        ident[0:65, 0:65],
                    )
                yield
                rr = outp.tile([128, 4, 1], F32, tag="rr")
                nc.vector.reciprocal(out=rr[:], in_=ob[:, :, D : D + 1])
                o_sb = outp.tile([128, 4, D], F32, tag="osb")
                nc.vector.tensor_tensor(
                    out=o_sb[:],
                    in0=ob[:, :, 0:D],
                    in1=rr[:, :, 0:1].to_broadcast([128, 4, D]),
                    op=mybir.AluOpType.mult,
                )
                nc.sync.dma_start(
                    out=out_h[h, 512 * ihalf : 512 * (ihalf + 1), :].rearrange(
                        "(i p) d -> p i d", p=128
                    ),
                    in_=o_sb[:],
                )
                yield

        for rep in range(replay):
            prev_pv = iter(())
            ld = emit_loads(0)
            emit_adj_prep()
            tr = emit_transposes(ld[0], ld[1])
            next(tr)
            kt_qt = next(tr)
            vp = ld[2]
            for h in range(H):
                front = emit_windows(h, *kt_qt)
                pt = next(front)
                nxt_ld = None
                nxt_tr = None
                nxt_kt_qt = None
                w = 0
                for _ in front:
                    w += 1
                    next(prev_pv, None)
                    if h + 1 < H:
                        if w == 4:
                            nxt_ld = emit_loads(h + 1)
                        elif w == 8:
                            nxt_tr = emit_transposes(nxt_ld[0], nxt_ld[1])
                            next(nxt_tr)
                        elif w == 12:
                            nxt_kt_qt = next(nxt_tr)
                    next(prev_pv, None)
                for _ in prev_pv:
                    pass
                prev_pv = emit_pv(h, pt, vp, last_mask["ins"])
                if h + 1 < H:
                    kt_qt = nxt_kt_qt
                    vp = nxt_ld[2]
            for _ in prev_pv:
                pass

    nc.compile()
    _CACHED_NC[replay] = nc
    return nc


def shard_inputs(queries, keys, values, adj):
    """Per-core input dicts: core c -> (batch c%4, query half c//4)."""
    in_maps = []
    for c in range(NCORES):
        b, qh = c % B, c // B
        in_maps.append(
            {
                "q_bh": np.ascontiguousarray(queries[b, :, qh * QH : (qh + 1) * QH, :]),
                "k_bh": np.ascontiguousarray(keys[b]),
                "v_bh": np.ascontiguousarray(values[b]),
                "adj_s": np.ascontiguousarray(adj[qh * QH : (qh + 1) * QH, :]),
            }
        )
    return in_maps


def assemble_output(results):
    h_prime = np.empty((B, H, N, D), dtype=np.float32)
    for c in range(NCORES):
        b, qh = c % B, c // B
        h_prime[b, :, qh * QH : (qh + 1) * QH, :] = results[c]["out"]
    return h_prime.reshape(N, B, H, D)


def kernel(queries, keys, values, adj):
    queries = np.asarray(queries, dtype=np.float32)
    keys = np.asarray(keys, dtype=np.float32)
    values = np.asarray(values, dtype=np.float32)
    adj = np.asarray(adj, dtype=np.int32)

    from concourse.bass_utils import run_bass_kernel_spmd

    nc = build_nc()
    res = run_bass_kernel_spmd(
        nc, shard_inputs(queries, keys, values, adj), core_ids=list(range(NCORES))
    )
    return assemble_output(res.results)
